# revision 76
# baseline (speedup 1.0000x reference)
# Trainium2 Bass kernel for nn_FFF_v2 (fast-feedforward / MoE tree routing).
#
#   lam   = x @ W.T                      [B, 12] router logits
#   branch= lam > 0                      tree descent decisions
#   node  = (2^i - 1) + sum_{j<i} branch_j 2^(i-1-j)
#   out   = sum_i lam_i * Y[node_i]      [B, 4096]
#
# Sharding: data-parallel on batch across 8 cores (1024 rows each); W and Y
# replicated.  Per core:
#   - router logits via PE matmul (fp32 -- sign fidelity vs the fp32
#     reference requires full precision; fp32r is tf32-ish)
#   - tree-node ids via small f32r matmuls (exact for integers < 2^12)
#   - levels 0..K_MM-1: scaled-one-hot bf16 matmul against SBUF-resident
#     shallow Y rows (1 cyc/row on PE)
#   - levels K_MM..11: dma_gather of bf16 Y rows from HBM + FMA chain
#     split across scalar + vector engines
#   - index replication for the 8 Q7 descriptor-gen cores via a PE
#     replication matmul (16 -> 128 partitions), no DRAM bounce
import numpy as np

DEPTH = 12
B = 8192
D = 4096
N_NODES = 4095
NCORES = 8
B_LOC = B // NCORES          # 1024 rows per core

MACRO = 256                  # batch rows per macro tile
SUB = 128                    # rows per subtile (one partition block)
NSUB = MACRO // SUB          # 2
NMACRO = B_LOC // MACRO      # 4

K_MM = 9                     # levels 0..K_MM-1 handled by one-hot matmul
N_SH = 2 ** K_MM - 1         # shallow nodes (511)
NCHUNK = (N_SH + 127) // 128  # 4
N_GL = DEPTH - K_MM          # gather levels (3)

_CACHE = {}


def _level_of(n):
    # level i spans nodes [2^i - 1, 2^(i+1) - 1)
    lev = 0
    while n >= 2 ** (lev + 1) - 1:
        lev += 1
    return lev


def _host_consts():
    # powT[j, i] = powmat[i, j] = 2^(i-1-j) for j < i  (lhsT of prefix matmul)
    powT = np.zeros((DEPTH, DEPTH), np.float32)
    for i in range(DEPTH):
        for j in range(i):
            powT[j, i] = float(1 << (i - 1 - j))
    # offsd[p, l*8+f] = 2^(K_MM+l) - 1 for deep levels, on all 128 partitions
    offsd = np.zeros((128, N_GL * (SUB // 16)), np.float32)
    for l in range(N_GL):
        offsd[:, l * (SUB // 16) : (l + 1) * (SUB // 16)] = float(
            (1 << (K_MM + l)) - 1
        )
    # bselT[l, c*128+p] = 1 if level(c*128+p) == l else 0   (lhsT of bc matmul)
    bselT = np.zeros((DEPTH, NCHUNK * 128), np.float32)
    # nrel[p, c] = node - (2^level - 1), or -1 for pad positions
    nrel = np.full((128, NCHUNK), -1.0, np.float32)
    for c in range(NCHUNK):
        for p in range(128):
            n = c * 128 + p
            if n < N_SH:
                lev = _level_of(n)
                bselT[lev, c * 128 + p] = 1.0
                nrel[p, c] = float(n - ((1 << lev) - 1))
    # identity for PE transposes of lam/prefix tiles
    ident = np.eye(2 * DEPTH, dtype=np.float32)
    # replT[k, m] = 1 if k == m % 16: PE matmul that replicates a
    # 16-partition tile across all 8 groups of 16 partitions
    replT = np.tile(np.eye(16, dtype=np.float32), (1, 8))
    eyem = np.eye(128, dtype=np.float32)
    # const gather indices for the shallow-Y load via the swdge pool:
    # yidx[c, 16g+p, f] = c*128 + f*16 + p (wrapped, replicated per group)
    yidx = np.zeros((NCHUNK, 128, SUB // 16), np.int16)
    for c in range(NCHUNK):
        for p in range(128):
            for f in range(SUB // 16):
                yidx[c, p, f] = c * 128 + f * 16 + (p % 16)
    return powT, offsd, bselT, nrel, ident, replT, eyem, yidx


def _build_program():
    import concourse.bass as bass
    import concourse.bacc as bacc
    import concourse.mybir as mybir
    import concourse.tile as tile
    from contextlib import ExitStack

    dt = mybir.dt
    f32 = dt.float32
    f32r = dt.float32r
    bf16 = dt.bfloat16
    i16 = dt.int16
    Alu = mybir.AluOpType

    nc = bacc.Bacc(trn_type="TRN2", num_swdge_queues=4)

    f16 = dt.float16
    ydt = bf16  # Y payload: bf16 halves gather bytes + 4x one-hot matmul rate
    # Router runs as 3 fp16 matmul groups (x_h@W_h + x_l@W_h + x_h@W_l):
    # fp16 hi/lo split reproduces fp32 logits to ~1e-5 (dropped x_l@W_l term)
    # at 1 cyc/row instead of fp32's LOW_HIGH 2-instruction path.
    xt_d = nc.dram_tensor("xt", [NMACRO, 128, 64, MACRO], f16, kind="ExternalInput")
    y_d = nc.dram_tensor("y", [N_NODES, D], ydt, kind="ExternalInput")
    wt_d = nc.dram_tensor("wt", [128, 96, DEPTH], f16, kind="ExternalInput")
    powt_d = nc.dram_tensor("powt", [DEPTH, DEPTH], f32r, kind="ExternalInput")
    offsd_d = nc.dram_tensor(
        "offsd", [128, N_GL * (SUB // 16)], f32, kind="ExternalInput"
    )
    bselt_d = nc.dram_tensor("bselt", [DEPTH, NCHUNK * 128], f32r, kind="ExternalInput")
    nrel_d = nc.dram_tensor("nrel", [128, NCHUNK], f32, kind="ExternalInput")
    ident_d = nc.dram_tensor("ident", [2 * DEPTH, 2 * DEPTH], f32r, kind="ExternalInput")
    replt_d = nc.dram_tensor("replt", [16, 128], f32r, kind="ExternalInput")
    eyem_d = nc.dram_tensor("eyem", [128, 128], bf16, kind="ExternalInput")
    yidx_d = nc.dram_tensor(
        "yidx", [NCHUNK, 128, SUB // 16], i16, kind="ExternalInput"
    )
    out_d = nc.dram_tensor("out", [B_LOC, D], bf16, kind="ExternalOutput")

    with tile.TileContext(nc) as tc, ExitStack() as ctx:
        consts = ctx.enter_context(tc.tile_pool(name="consts", bufs=1))
        xt_p = ctx.enter_context(tc.tile_pool(name="xt", bufs=2))
        small = ctx.enter_context(tc.tile_pool(name="small", bufs=3))
        small4 = ctx.enter_context(tc.tile_pool(name="small4", bufs=6))
        st_p = ctx.enter_context(tc.tile_pool(name="st", bufs=8))
        g_p = ctx.enter_context(tc.tile_pool(name="g", bufs=5))
        acc_p = ctx.enter_context(tc.tile_pool(name="acc", bufs=2))
        scr_p = ctx.enter_context(tc.tile_pool(name="scr", bufs=1))
        out_p = ctx.enter_context(tc.tile_pool(name="outp", bufs=2))
        ps_lam = ctx.enter_context(tc.tile_pool(name="pslam", bufs=1, space="PSUM"))
        ps_pb = ps_lam
        ps_bc = ctx.enter_context(tc.tile_pool(name="psbc", bufs=2, space="PSUM"))
        ps_tp = ctx.enter_context(tc.tile_pool(name="pstp", bufs=2, space="PSUM"))
        ps_out = ctx.enter_context(tc.tile_pool(name="psout", bufs=2, space="PSUM"))

        # ---- critical-path constants: router weights + shallow Y rows,
        # split across engine DMA queues so no single ~25 GB/s queue
        # serializes the startup ----
        # yidx first (tiny, unblocks the ysh gathers); router weights on the
        # scalar queue so the first x_h tile leads the sync queue
        yidx_sb = consts.tile([128, NCHUNK, SUB // 16], i16)
        nc.sync.dma_start(
            yidx_sb[:], yidx_d.ap().rearrange("c p f -> p c f")
        )
        wt_sb = consts.tile([128, 96, DEPTH], f16)
        nc.scalar.dma_start(wt_sb[:], wt_d.ap())
        ysh = []
        for c in range(NCHUNK):
            yc = consts.tile([128, 1, D], ydt, tag=f"yc{c}")
            nc.gpsimd.dma_gather(
                yc[:], y_d.ap(), yidx_sb[:, c, :], SUB, SUB, D, queue_num=c % 4
            )
            ysh.append(yc)

        for m in range(NMACRO):
            # ---- load x^T macro tile [128, 64, MACRO] (fp16 hi | lo),
            # halves on different engine queues ----
            xt = xt_p.tile([128, 64, MACRO], f16, tag="xt")
            nc.sync.dma_start(xt[:, :32, :], xt_d.ap()[m][:, :32, :])
            nc.scalar.dma_start(xt[:, 32:, :], xt_d.ap()[m][:, 32:, :])

            # ---- router: lam^T [12, MACRO] = W @ x^T via fp16 hi/lo ----
            # term 1: W_h @ x_h (wt chunks 0:32 on xt 0:32)
            # term 2: W_l @ x_h (wt chunks 32:64 on xt 0:32)
            # term 3: W_h @ x_l (wt chunks 64:96 on xt 32:64)
            lam_ps = ps_lam.tile([DEPTH, MACRO], f32, tag="lam")
            for c in range(96):
                xc = c if c < 32 else (c - 32 if c < 64 else c - 32)
                nc.tensor.matmul(
                    lam_ps[:], wt_sb[:, c, :], xt[:, xc, :],
                    start=(c == 0), stop=(c == 95),
                )

            if m == 0:
                # remaining constants, deferred so the first x tile and the
                # router aren't stuck behind const DMA traffic
                powt_sb = consts.tile([DEPTH, DEPTH], f32r)
                nc.sync.dma_start(powt_sb[:], powt_d.ap())
                offsd_sb = consts.tile([128, N_GL * (SUB // 16)], f32)
                nc.sync.dma_start(offsd_sb[:], offsd_d.ap())
                bselt_sb = consts.tile([DEPTH, NCHUNK * 128], f32r)
                nc.sync.dma_start(bselt_sb[:], bselt_d.ap())
                nrel_sb = consts.tile([128, NCHUNK], f32)
                nc.sync.dma_start(nrel_sb[:], nrel_d.ap())
                ident_sb = consts.tile([2 * DEPTH, 2 * DEPTH], f32r)
                nc.sync.dma_start(ident_sb[:], ident_d.ap())
                replt_sb = consts.tile([16, 128], f32r)
                nc.sync.dma_start(replt_sb[:], replt_d.ap())
                eyem_sb = consts.tile([128, 128], bf16)
                nc.sync.dma_start(eyem_sb[:], eyem_d.ap())

            # branch bits, lam^T and prefix^T in SBUF (partition 0 based)
            branch = small.tile([DEPTH, MACRO], f32r, tag="branch")
            nc.vector.tensor_scalar(branch[:], lam_ps[:], 0.0, None, Alu.is_gt)
            lamT = small.tile([DEPTH, MACRO], f32r, tag="lamT")
            nc.scalar.copy(lamT[:], lam_ps[:])

            # prefix^T [12, MACRO] = powmat @ branch  (f32r, exact ints)
            pb_ps = ps_pb.tile([DEPTH, MACRO], f32, tag="lam")
            nc.tensor.matmul(pb_ps[:], powt_sb[:], branch[:], start=True, stop=True)
            pfxT = small.tile([DEPTH, MACRO], f32r, tag="pfxT")
            nc.scalar.copy(pfxT[:], pb_ps[:])

            # ---- S^T build: one chunk of 128 shallow nodes at a time ----
            st = []
            for c in range(NCHUNK):
                bc_ps = ps_bc.tile([128, 2 * MACRO], f32, tag="bc")
                nc.tensor.matmul(
                    bc_ps[:, :MACRO], bselt_sb[:, c * 128 : (c + 1) * 128],
                    pfxT[:], start=True, stop=True,
                )
                nc.tensor.matmul(
                    bc_ps[:, MACRO:], bselt_sb[:, c * 128 : (c + 1) * 128],
                    lamT[:], start=True, stop=True,
                )
                lbc = small.tile([128, MACRO], f32, tag="lbc")
                nc.scalar.copy(lbc[:], bc_ps[:, MACRO:])
                stc = st_p.tile([128, MACRO], ydt, tag="st")
                nc.vector.scalar_tensor_tensor(
                    stc[:], bc_ps[:, :MACRO], nrel_sb[:, c : c + 1], lbc[:],
                    Alu.is_equal, Alu.mult,
                )
                st.append(stc)

            for s in range(NSUB):
                bsl = slice(s * SUB, (s + 1) * SUB)
                # ---- lam to batch-partition layout ----
                # (plain identity matmul: out = in.T @ I)
                tpw = ps_tp.tile([SUB, 160], f32, tag="tpw")
                tp_ps = tpw[:, :DEPTH]
                nc.tensor.matmul(
                    tp_ps, lamT[:, bsl], ident_sb[:DEPTH, :DEPTH],
                    start=True, stop=True,
                )
                lamb = small4.tile([SUB, DEPTH], f32, tag="lamb")
                nc.vector.tensor_copy(lamb[:], tp_ps)

                # ---- node ids: per-16-column PE transposes of prefix^T ----
                w_ps = tpw[:16, 16 : 16 + (SUB // 16) * DEPTH].rearrange(
                    "p (f l) -> p f l", f=SUB // 16
                )
                for f in range(SUB // 16):
                    nc.tensor.matmul(
                        w_ps[:, f, :],
                        pfxT[:, s * SUB + f * 16 : s * SUB + (f + 1) * 16],
                        ident_sb[:DEPTH, :DEPTH],
                        start=True, stop=True,
                    )
                # deep-level relative ids to SBUF, then replicate to all 8
                # 16-partition groups via the PE (no DRAM bounce)
                idx16f = small4.tile([16, SUB // 16, N_GL], f32r, tag="idx16f")
                nc.scalar.copy(idx16f[:], w_ps[:, :, K_MM:])
                rep_ps = tpw[:, 128 : 128 + (SUB // 16) * N_GL]
                nc.tensor.matmul(
                    rep_ps,
                    replt_sb[:],
                    idx16f[:].rearrange("p f l -> p (f l)"),
                    start=True, stop=True,
                )
                idxr = small4.tile([128, N_GL, SUB // 16], i16, tag="idxr")
                nc.vector.tensor_tensor(
                    idxr[:],
                    rep_ps.rearrange("p (f l) -> p f l", f=SUB // 16).rearrange(
                        "p f l -> p l f"
                    ),
                    offsd_sb[:].rearrange("p (l f) -> p l f", l=N_GL),
                    Alu.add,
                )

                # ---- gather deep levels from HBM (bf16 rows) ----
                gt = []
                for li in range(N_GL):
                    g = g_p.tile([128, 1, D], ydt, tag="g")
                    nc.gpsimd.dma_gather(
                        g[:], y_d.ap(), idxr[:, li, :],
                        SUB, SUB, D, queue_num=li % 4,
                    )
                    gt.append(g)

                # one-hot matmul: shallow-level contribution (bf16 PE), with
                # the PSUM q-tiles downcast to one full-width bf16 row on the
                # scalar engine
                po16f = acc_p.tile([SUB, D], bf16, tag="po16f")
                for q in range(D // 512):
                    po = ps_out.tile([SUB, 512], f32, tag="po")
                    for c in range(NCHUNK):
                        nc.tensor.matmul(
                            po[:], st[c][:, bsl],
                            ysh[c][:, 0, q * 512 : (q + 1) * 512],
                            start=(c == 0), stop=(c == NCHUNK - 1),
                        )
                    nc.scalar.copy(po16f[:, q * 512 : (q + 1) * 512], po[:])

                # full-width bf16 FMA tree: tensor_scalar (4x mode) products,
                # tensor_tensor (2x mode) adds
                out_t = out_p.tile([SUB, D], bf16, tag="out")
                t_a = scr_p.tile([SUB, D], bf16, tag="ta")
                t_b = scr_p.tile([SUB, D], bf16, tag="tb")
                nc.vector.tensor_scalar(
                    t_a[:], gt[0][:, 0, :], lamb[:, K_MM : K_MM + 1], None, Alu.mult
                )
                nc.vector.tensor_scalar(
                    t_b[:], gt[1][:, 0, :], lamb[:, K_MM + 1 : K_MM + 2], None,
                    Alu.mult,
                )
                nc.vector.tensor_tensor(out_t[:], t_a[:], t_b[:], Alu.add)
                nc.vector.tensor_scalar(
                    t_a[:], gt[2][:, 0, :], lamb[:, K_MM + 2 : K_MM + 3], None,
                    Alu.mult,
                )
                nc.vector.tensor_tensor(out_t[:], out_t[:], t_a[:], Alu.add)
                nc.vector.tensor_tensor(out_t[:], out_t[:], po16f[:], Alu.add)
                assert N_GL == 3
                nc.sync.dma_start(out_d.ap()[m * MACRO + s * SUB :][:SUB, :], out_t[:])

    nc.compile()
    return nc


def _patch_walrus_passes():
    # The default walrus pass list in this environment omits
    # lower_custom_kernel, which the Pool custom instructions (dma_gather)
    # need. Inject it in front of codegen.
    import concourse.bass_utils as bu

    if getattr(bu, "_ant_lck_patched", False):
        return
    bu._ant_lck_patched = True
    orig = bu.run_command

    def run_command(argv, **kw):
        if argv and "walrus_driver" in str(argv[0]):
            argv = list(argv)
            for i, a in enumerate(argv):
                if a == "--pass" and "lower_custom_kernel" not in argv[i + 1]:
                    argv[i + 1] = argv[i + 1].replace(
                        "codegen", "lower_custom_kernel,codegen"
                    )
                    break
        return orig(argv, **kw)

    bu.run_command = run_command


def _get_program():
    if "nc" not in _CACHE:
        _CACHE["nc"] = _build_program()
    return _CACHE["nc"]


def _prep_in_maps(x, W, Y):
    import ml_dtypes

    powT, offsd, bselT, nrel, ident, replT, eyem, yidx = _host_consts()
    eyem = eyem.astype(ml_dtypes.bfloat16)
    Y = np.ascontiguousarray(Y, np.float32).astype(ml_dtypes.bfloat16)
    # fp16 hi/lo split of W: wt chunks = [W_h (32) | W_l (32) | W_h (32)]
    Wf = np.ascontiguousarray(W, np.float32)
    W_h = Wf.astype(np.float16)
    W_l = (Wf - W_h.astype(np.float32)).astype(np.float16)
    w_h = W_h.T.reshape(32, 128, DEPTH)
    w_l = W_l.T.reshape(32, 128, DEPTH)
    wt = np.ascontiguousarray(
        np.concatenate([w_h, w_l, w_h], axis=0).transpose(1, 0, 2), np.float16
    )
    in_maps = []
    xr = x.reshape(NCORES, B_LOC, D)
    for c in range(NCORES):
        xt = np.ascontiguousarray(xr[c].T, np.float32)  # [D, B_LOC]
        x_h = xt.astype(np.float16)
        x_l = (xt - x_h.astype(np.float32)).astype(np.float16)
        xtm = np.ascontiguousarray(
            np.concatenate(
                [
                    x_h.reshape(32, 128, NMACRO, MACRO),
                    x_l.reshape(32, 128, NMACRO, MACRO),
                ],
                axis=0,
            ).transpose(2, 1, 0, 3),
            np.float16,
        )
        in_maps.append(
            {
                "xt": xtm, "y": Y, "wt": wt, "powt": powT, "offsd": offsd,
                "bselt": bselT, "nrel": nrel, "ident": ident, "replt": replT,
                "eyem": eyem, "yidx": yidx,
            }
        )
    return in_maps


def kernel(x, W, Y, _trace=False):
    from concourse.bass_utils import run_bass_kernel_spmd

    _patch_walrus_passes()

    nc = _get_program()
    in_maps = _prep_in_maps(np.asarray(x), np.asarray(W), np.asarray(Y))
    res = run_bass_kernel_spmd(nc, in_maps, list(range(NCORES)), trace=_trace)
    out = np.concatenate(
        [np.asarray(res.results[c]["out"]).astype(np.float32) for c in range(NCORES)],
        axis=0,
    )
    if _trace:
        _CACHE["last_result"] = res
    return out
